# revision 28
# baseline (speedup 1.0000x reference)
"""AQT-style int8 dot_general (quantize -> int matmul -> dequant) on 8 TRN2 cores.

Sharding: 4x2 (M x N) tensor-parallel grid, K unsharded -> no collectives.
Each core: lhs [2048, 4096] row-shard, rhs [4096, 2048] col-shard.

Per core, N is processed in two 1024-wide halves so the quantized rhs half
(bf16, 8MB) stays SBUF-resident while leaving room for the lhs pipeline:

  lhs pipeline (independent, starts at t=0): per 128-row m-tile, per-row
      absmax -> x*(127.5/absmax) on ACT -> +C/-C RNE round to bf16 -> DRAM
      scratch; matmul panels are XBAR-transpose-loaded [K,M] from scratch.
  per half: A) stream rhs, per-column absmax (ACT abs + DVE max chain +
      gpsimd partition_all_reduce); B) re-stream rhs, quantize to resident
      bf16 tiles; C) per m-tile: 32 accumulating matmuls per 512-wide psum
      chunk, fused dequant (acc * s_l[m] * s_r[n]) on eviction.

Numerics: quantized values are exact small ints in bf16; products accumulate
exactly in fp32 PSUM (|acc| << 2^24). round() is the +C/-C fp32 RNE trick with
the clip folded into a 127.5*(1-2^-21) multiplier (the abs-max element lands
just below 127.5 -> rounds to 127, matching the reference's clip(128)->127).
"""

import numpy as np

import concourse.bass as bass
import concourse.tile as tile
from concourse import bacc, bass_isa, mybir
from concourse.bass import ds, ts
from concourse.bass_utils import run_bass_kernel_spmd

M_FULL, K_FULL, N_FULL = 8192, 4096, 4096
GM, GN = 4, 2
N_CORES = GM * GN
P = 128

F32 = mybir.dt.float32
BF16 = mybir.dt.bfloat16

C_MAGIC = 12582912.0  # 1.5 * 2^23: +C then -C rounds fp32 to nearest int (RNE)
QSCALE = 127.5 * (1.0 - 2.0**-21)
INV_CLIP = float(np.float32(1.0) / np.float32(127.5))


def build_nc(
    msh=M_FULL // GM,
    nsh=N_FULL // GN,
    k=K_FULL,
    n_cores=N_CORES,
    mt_limit=None,
):
    kt_n = k // P  # 32 k-tiles
    mt_n = msh // P if mt_limit is None else mt_limit  # 16 m-tiles
    nh = nsh // 2  # half width (1024)
    nfree = 512 if nh % 512 == 0 else nh
    nch_n = nh // nfree  # psum chunks per half
    kh = k // 2  # lhs chunk width

    nc = bacc.Bacc("TRN2", target_bir_lowering=False, debug=False, num_devices=n_cores)
    lhs = nc.dram_tensor("lhs", [msh, k], F32, kind="ExternalInput").ap()
    rhs = nc.dram_tensor("rhs", [k, nsh], F32, kind="ExternalInput").ap()
    out = nc.dram_tensor("out", [msh, nsh], F32, kind="ExternalOutput").ap()

    with tile.TileContext(nc) as tc:
        with (
            tc.tile_pool(name="stream", bufs=3) as stream,
            tc.tile_pool(name="qrhs", bufs=kt_n + 8) as qrhsp,
            tc.tile_pool(name="scale", bufs=2) as scalep,
            tc.tile_pool(name="small", bufs=1) as smallp,
            tc.tile_pool(name="lstr", bufs=4) as lstrp,
            tc.tile_pool(name="lsm", bufs=4) as lsmp,
            tc.tile_pool(name="qm", bufs=2) as qmp,
            tc.tile_pool(name="panel", bufs=3) as panelp,
            tc.tile_pool(name="evict", bufs=3) as evictp,
            tc.tile_pool(name="dram", bufs=mt_n if mt_n else 1, space="DRAM") as dramp,
            tc.tile_pool(name="psum", bufs=8, space="PSUM") as psump,
        ):
            halves = []  # (rq, sr) per half
            # 4-core column groups (cores c = ni*GM + mi share the same rhs
            # shard); each computes absmax over its K/4 rows, AllReduce-max
            split_a = n_cores == GM * GN and kt_n % GM == 0
            rgroups = [
                list(range(g * GM, (g + 1) * GM)) for g in range(GN)
            ]

            def phase_a(h):
                pmax = scalep.tile([P, nh], F32, tag="pmax")
                nc.vector.memset(pmax[:], 0.0)
                if split_a:
                    pid = nc.sync.partition_id()
                    row0 = (pid % GM) * (k // GM)
                    local_kt = kt_n // GM
                else:
                    row0 = 0
                    local_kt = kt_n
                for kt in range(local_kt):
                    rt = stream.tile([P, nh], F32, tag="rt")
                    if split_a:
                        src_ap = rhs[ds(row0 + kt * P, P), ds(h * nh, nh)]
                    else:
                        src_ap = rhs[ts(kt, P), ds(h * nh, nh)]
                    nc.sync.dma_start(rt[:], src_ap)
                    nc.scalar.activation(
                        rt[:], rt[:], mybir.ActivationFunctionType.Abs
                    )
                    nc.vector.tensor_tensor(
                        pmax[:], pmax[:], rt[:], mybir.AluOpType.max
                    )
                sr = scalep.tile([P, nh], F32, tag="sr")
                nc.gpsimd.partition_all_reduce(
                    sr[:], pmax[:], channels=P, reduce_op=bass_isa.ReduceOp.absmax
                )
                if split_a:
                    ccin = dramp.tile([1, nh], F32, name=f"ccin{h}")
                    ccout = dramp.tile([1, nh], F32, name=f"ccout{h}")
                    nc.sync.dma_start(ccin[:], sr[0:1, :])
                    nc.gpsimd.collective_compute(
                        "AllReduce",
                        mybir.AluOpType.max,
                        replica_groups=rgroups,
                        ins=[ccin[:]],
                        outs=[ccout[:]],
                    )
                    nc.sync.dma_start(sr[0:1, :], ccout[:])
                    nc.gpsimd.partition_broadcast(sr[:], sr[0:1, :])
                rq = scalep.tile([P, nh], F32, tag="rq")
                nc.vector.reciprocal(rq[:], sr[:])
                nc.vector.tensor_scalar_mul(rq[:], rq[:], QSCALE)
                # dequant scale s_r = absmax/127.5 (in place; absmax dead)
                nc.vector.tensor_scalar_mul(sr[:], sr[:], INV_CLIP)
                return rq, sr

            def phase_b(h, rq):
                q_tiles = []
                for kt in range(kt_n):
                    rt = stream.tile([P, nh], F32, tag="rt")
                    nc.sync.dma_start(rt[:], rhs[ts(kt, P), ds(h * nh, nh)])
                    nc.vector.tensor_mul(rt[:], rt[:], rq[:])
                    q = qrhsp.tile([P, nh], BF16)
                    nc.vector.tensor_scalar(
                        q[:],
                        rt[:],
                        C_MAGIC,
                        C_MAGIC,
                        mybir.AluOpType.add,
                        mybir.AluOpType.subtract,
                    )
                    q_tiles.append(q)
                return q_tiles

            # ---- half 0 absmax ----
            halves.append(phase_a(0))

            # ---- lhs quantize pipeline (first tiles outrank phase B) ----
            s_l = smallp.tile([P, max(mt_n, 1)], F32)
            qdram = [
                dramp.tile([P, k], BF16, name=f"qd{mt}") for mt in range(mt_n)
            ]
            q_half = None

            for mt in range(mt_n):
                if mt == 6 and q_half is None:
                    q_half = phase_b(0, halves[0][0])
                chunks = []
                pa = lsmp.tile([P, 2], F32, tag="pa")
                for hh in range(2):
                    lc = lstrp.tile([P, kh], F32, tag="lc")
                    nc.sync.dma_start(lc[:], lhs[ts(mt, P), ds(hh * kh, kh)])
                    nc.vector.tensor_reduce(
                        pa[:, hh : hh + 1],
                        lc[:],
                        axis=mybir.AxisListType.X,
                        op=mybir.AluOpType.max,
                        apply_absolute_value=True,
                    )
                    chunks.append(lc)
                am = lsmp.tile([P, 1], F32, tag="am")
                nc.vector.tensor_reduce(
                    am[:], pa[:], axis=mybir.AxisListType.X, op=mybir.AluOpType.max
                )
                rql = lsmp.tile([P, 1], F32, tag="rql")
                nc.vector.reciprocal(rql[:], am[:])
                nc.vector.tensor_scalar_mul(rql[:], rql[:], QSCALE)
                nc.vector.tensor_scalar_mul(s_l[:, mt : mt + 1], am[:], INV_CLIP)
                for hh in range(2):
                    # rql*x + C rounds to integer+C in fp32 (RNE); then -C on
                    # the second pass emits exact small ints as bf16
                    nc.scalar.activation(
                        chunks[hh][:],
                        chunks[hh][:],
                        mybir.ActivationFunctionType.Copy,
                        scale=rql[:],
                        bias=C_MAGIC,
                    )
                    qmt = qmp.tile([P, kh], BF16)
                    nc.scalar.activation(
                        qmt[:],
                        chunks[hh][:],
                        mybir.ActivationFunctionType.Copy,
                        bias=-C_MAGIC,
                    )
                    nc.sync.dma_start(qdram[mt][:, ds(hh * kh, kh)], qmt[:])

            if q_half is None:
                q_half = phase_b(0, halves[0][0])

            # ---- half 1 rhs prep (overlaps half-0 matmuls) ----
            halves.append(phase_a(1))

            def evict_store(h, mt, nci, ps, sr):
                ev = evictp.tile([P, nfree], F32, tag="ev", name=f"ev{h}_{mt}_{nci}")
                nc.vector.scalar_tensor_tensor(
                    ev[:],
                    ps[:],
                    s_l[:, mt : mt + 1],
                    sr[:, ds(nci * nfree, nfree)],
                    op0=mybir.AluOpType.mult,
                    op1=mybir.AluOpType.mult,
                )
                nc.gpsimd.dma_start(
                    out[ts(mt, P), ds(h * nh + nci * nfree, nfree)], ev[:]
                )

            def mloop(h, rq_sr, q_tiles):
                rq, sr = rq_sr
                # interleaved head: first `ilv` m-tiles share the k-loop so the
                # PE consumes each q_rhs k-tile as soon as phase B produces it
                ilv = min(mt_n, 8 // max(nch_n, 1))
                if ilv > 1:
                    panels = []
                    for mt in range(ilv):
                        panel = panelp.tile(
                            [P, kt_n, P], BF16, tag="panel", name=f"hpan{h}_{mt}"
                        )
                        nc.scalar.dma_start_transpose(panel[:], qdram[mt][:])
                        panels.append(panel)
                    pss = [
                        [
                            psump.tile([P, nfree], F32, tag="ps", name=f"hps{h}_{mt}_{nci}")
                            for nci in range(nch_n)
                        ]
                        for mt in range(ilv)
                    ]
                    for kc in range(kt_n):
                        for mt in range(ilv):
                            for nci in range(nch_n):
                                nc.tensor.matmul(
                                    pss[mt][nci][:],
                                    panels[mt][:, kc, :],
                                    q_tiles[kc][:, ds(nci * nfree, nfree)],
                                    start=(kc == 0),
                                    stop=(kc == kt_n - 1),
                                )
                    for mt in range(ilv):
                        for nci in range(nch_n):
                            evict_store(h, mt, nci, pss[mt][nci], sr)
                for mt in range(ilv, mt_n):
                    # panel[p, c, m] = q_lhs[mt*128+m, c*128+p]
                    panel = panelp.tile([P, kt_n, P], BF16, tag="panel")
                    nc.scalar.dma_start_transpose(panel[:], qdram[mt][:])
                    for nci in range(nch_n):
                        ps = psump.tile([P, nfree], F32, tag="ps")
                        for kc in range(kt_n):
                            nc.tensor.matmul(
                                ps[:],
                                panel[:, kc, :],
                                q_tiles[kc][:, ds(nci * nfree, nfree)],
                                start=(kc == 0),
                                stop=(kc == kt_n - 1),
                            )
                        evict_store(h, mt, nci, ps, sr)

            q_half1 = phase_b(1, halves[1][0])
            mloop(0, halves[0], q_half)
            mloop(1, halves[1], q_half1)
    nc.compile()
    return nc


_NC_CACHE = {}


def _get_nc():
    if "nc" not in _NC_CACHE:
        _NC_CACHE["nc"] = build_nc()
    return _NC_CACHE["nc"]


def kernel(lhs, rhs):
    lhs = np.ascontiguousarray(np.asarray(lhs), dtype=np.float32)
    rhs = np.ascontiguousarray(np.asarray(rhs), dtype=np.float32)
    assert lhs.shape == (M_FULL, K_FULL) and rhs.shape == (K_FULL, N_FULL)
    msh, nsh = M_FULL // GM, N_FULL // GN
    nc = _get_nc()
    in_maps = []
    for c in range(N_CORES):
        mi, ni = c % GM, c // GM
        in_maps.append(
            {
                "lhs": np.ascontiguousarray(lhs[mi * msh : (mi + 1) * msh, :]),
                "rhs": np.ascontiguousarray(rhs[:, ni * nsh : (ni + 1) * nsh]),
            }
        )
    res = run_bass_kernel_spmd(nc, in_maps, core_ids=list(range(N_CORES)))
    outp = np.empty((M_FULL, N_FULL), dtype=np.float32)
    for c in range(N_CORES):
        mi, ni = c % GM, c // GM
        outp[mi * msh : (mi + 1) * msh, ni * nsh : (ni + 1) * nsh] = res.results[c][
            "out"
        ]
    return outp
